# revision 1
# baseline (speedup 1.0000x reference)
"""COO SpMM (GNN message passing) on 8 Trainium2 NeuronCores.

out[b, d] = sum_e vals[e] * x[b, col[e]]  for row[e]==d,  + bias[d]

Strategy (dst-sharded, batched dma_gather, bf16):
  - Sources are split into two windows (A/B) so edge indices fit the
    int16 index range of dma_gather (signed offsets from a per-call
    in_ap base give a 64K-row reach per window; rows at idx==-1 are
    reserved zero rows).
  - Dsts are ranked by (degree desc, A-count desc) so the 128-dst
    blocks have near-uniform per-window degree profiles; global block
    g -> core g%8 keeps the shared SPMD schedule's padding small.
  - Per local block bl: L_A[bl] columns for window-A edges and L_B[bl]
    for window-B (max over the 8 cores).  Columns are laid out in
    superblocks of SB blocks: [A-cols of SB blocks][B-cols of ...] so
    each window run is one dma_gather call (~100 columns = ~13K rows
    gathered per call; the ~1us SWDGE fixed cost is amortized 100x).
  - Gathered rows are 128 bf16 (= 256B, dma_gather's minimum; cols
    64..127 are zero).  One broadcast tensor_tensor per call scales
    all columns by their edge weights (bf16, 2x DVE).  Per column a
    bf16 identity matmul accumulates the scaled [128, 64] slice into
    the block's fp32 PSUM accumulator; block done -> bias add -> DMA.
  - Host: un-permute ranks, transpose back to [64, 100000] f32.
"""
import sys
import numpy as np

sys.path.insert(0, "/opt/trn_rl_repo")

NUM_SRC = 100000
NUM_DST = 100000
NNZ = 3200000
BATCH = 64
NCORES = 8
P = 128
ROWW = 128                       # bf16 elements per table row (256B)
NBLK = 98                        # local blocks per core
NBLK_G = NBLK * NCORES           # 784 global blocks
DTOT = NBLK_G * P                # 100352 ranked dst slots
SB = 6                           # blocks per superblock
MAXC = 96                        # max columns per dma_gather call
HALF = 32768


def _preprocess(row, col, vals):
    import ml_dtypes
    bf16 = ml_dtypes.bfloat16

    row = np.asarray(row).astype(np.int64)
    col = np.asarray(col).astype(np.int64)
    vals = np.asarray(vals).astype(np.float32)

    # --- source -> window (A=0 / B=1): greedy per-dst balance -------------
    deg0 = np.bincount(row, minlength=NUM_DST)
    o = np.argsort(col, kind="stable")
    dsts_by_src = row[o]
    sdeg = np.bincount(col, minlength=NUM_SRC)
    indptr = np.concatenate([[0], np.cumsum(sdeg)])
    order_src = np.argsort(-sdeg, kind="stable")
    ip0, ip1 = indptr[:-1], indptr[1:]
    D = np.zeros(NUM_DST, np.int32)
    wsrc = np.zeros(NUM_SRC, np.int8)
    for s in order_src:
        ds = dsts_by_src[ip0[s]:ip1[s]]
        if D[ds].sum() >= 0:
            wsrc[s] = 1
            D[ds] -= 1
        else:
            D[ds] += 1
    sgn = {0: 1, 1: -1}
    for _ in range(2):
        for s in order_src:
            ds = dsts_by_src[ip0[s]:ip1[s]]
            c = sgn[wsrc[s]]
            w_new = 1 if (D[ds].sum() - c * len(ds)) >= 0 else 0
            if w_new != wsrc[s]:
                D[ds] += sgn[w_new] - c
                wsrc[s] = w_new
    a_src = np.where(wsrc == 0)[0]
    b_src = np.where(wsrc == 1)[0]
    nA_src, nB_src = len(a_src), len(b_src)
    # window A rows [0, A_end) skipping reserved row HALF-1
    a_rows = np.arange(nA_src, dtype=np.int64)
    if nA_src > HALF - 1:
        a_rows[a_rows >= HALF - 1] += 1
    A_end = int(a_rows[-1]) + 1 if nA_src else 0
    base_A = HALF
    base_B = A_end + HALF
    b_rows = A_end + np.arange(nB_src, dtype=np.int64)
    if nB_src > HALF - 1:
        b_rows[b_rows >= base_B - 1] += 1
    B_end = int(b_rows[-1]) + 1 if nB_src else A_end
    NTAB = max(B_end, base_B + 1)
    rowof = np.empty(NUM_SRC, np.int64)
    rowof[a_src] = a_rows
    rowof[b_src] = b_rows

    # --- dst ranking ------------------------------------------------------
    deg = np.bincount(row, minlength=NUM_DST)
    w_e = wsrc[col].astype(np.int64)
    nAd = np.bincount(row[w_e == 0], minlength=NUM_DST)
    order_dst = np.lexsort((-nAd, -deg))          # rank -> dst id
    rank_of = np.empty(NUM_DST, np.int64)
    rank_of[order_dst] = np.arange(NUM_DST)

    # --- per-block column counts (shared schedule) ------------------------
    nApad = np.zeros(DTOT, np.int64)
    nApad[:NUM_DST] = nAd[order_dst]
    nBpad = np.zeros(DTOT, np.int64)
    nBpad[:NUM_DST] = (deg - nAd)[order_dst]
    L_A = nApad.reshape(NBLK_G, P).max(1).reshape(NBLK, NCORES).max(1)
    L_B = nBpad.reshape(NBLK_G, P).max(1).reshape(NBLK, NCORES).max(1)
    L_A[(L_A + L_B) == 0] = 1

    # --- column layout: per superblock [A-cols][B-cols] -------------------
    colbase_A = np.zeros(NBLK, np.int64)
    colbase_B = np.zeros(NBLK, np.int64)
    calls = []                                    # (start, ncols, window)

    def _add_calls(s, n, w):
        while n > 0:
            c = min(n, MAXC)
            calls.append((s, c, w))
            s += c
            n -= c

    cptr = 0
    for sb0 in range(0, NBLK, SB):
        bls = range(sb0, min(sb0 + SB, NBLK))
        sA = cptr
        for bl in bls:
            colbase_A[bl] = cptr
            cptr += int(L_A[bl])
        _add_calls(sA, cptr - sA, 0)
        sB = cptr
        for bl in bls:
            colbase_B[bl] = cptr
            cptr += int(L_B[bl])
        _add_calls(sB, cptr - sB, 1)
    ngroups = cptr

    blk_of_g = np.zeros(ngroups, np.int64)
    for bl in range(NBLK):
        blk_of_g[colbase_A[bl]:colbase_A[bl] + L_A[bl]] = bl
        blk_of_g[colbase_B[bl]:colbase_B[bl] + L_B[bl]] = bl
    first_of = np.where(L_A > 0, colbase_A, colbase_B)
    last_of = np.where(L_B > 0, colbase_B + L_B - 1, colbase_A + L_A - 1)

    # --- per-edge schedule ------------------------------------------------
    r = rank_of[row]
    key = r * 2 + w_e
    order = np.argsort(key, kind="stable")
    k_s = key[order]
    col_s = col[order]
    vals_s = vals[order]
    counts = np.bincount(k_s, minlength=DTOT * 2)
    starts = np.concatenate([[0], np.cumsum(counts)[:-1]])
    j_s = np.arange(len(k_s)) - starts[k_s]
    r_s = k_s // 2
    w_s = k_s % 2
    gb_s = r_s // P
    slot_s = r_s % P
    core_s = gb_s % NCORES
    bl_s = gb_s // NCORES
    g_s = np.where(w_s == 0, colbase_A[bl_s], colbase_B[bl_s]) + j_s
    idx_s = rowof[col_s] - np.where(w_s == 0, base_A, base_B)
    assert idx_s.min() >= -HALF and idx_s.max() < HALF and not (idx_s == -1).any()

    per_core = []
    for k in range(NCORES):
        m = core_s == k
        idxmat = np.zeros((P, ngroups), dtype=np.int16)
        valsw = np.zeros((P, ngroups), dtype=bf16)
        idxmat[slot_s[m], g_s[m]] = idx_s[m].astype(np.int16)
        valsw[slot_s[m], g_s[m]] = vals_s[m].astype(bf16)
        # wrap idx into the 16-partition dma_gather layout, per call;
        # append one idx=0 pad column per call — the ucode silently drops a
        # TRAILING run of negative indices, so the list must end >= 0.
        parts = []
        for (start, n, w) in calls:
            sub = np.concatenate(
                [idxmat[:, start:start + n], np.zeros((P, 1), np.int16)], axis=1)
            wrapped = sub.reshape(8, 16, n + 1).transpose(1, 2, 0).reshape(
                16, 8 * (n + 1))
            parts.append(np.tile(wrapped, (8, 1)))
        idxall = np.concatenate(parts, axis=1)
        per_core.append((idxall, valsw))

    sched = {
        "calls": tuple(calls),
        "ngroups": ngroups,
        "NTAB": NTAB,
        "base_A": base_A,
        "base_B": base_B,
        "blk_of_g": blk_of_g,
        "first_of": first_of,
        "last_of": last_of,
        "rowof": rowof,
    }
    return sched, per_core, order_dst


_BUILD_CACHE = {}


def _build(sched):
    key = (sched["ngroups"], sched["NTAB"], sched["calls"])
    if key in _BUILD_CACHE:
        return _BUILD_CACHE[key]
    import concourse.bacc as bacc
    import concourse.mybir as mybir
    import concourse.tile as tile
    from concourse.masks import make_identity

    ngroups = sched["ngroups"]
    NTAB = sched["NTAB"]
    bases = (sched["base_A"], sched["base_B"])
    blk_of_g = sched["blk_of_g"]
    first_of = sched["first_of"]
    last_of = sched["last_of"]

    nc = bacc.Bacc("TRN2", target_bir_lowering=False, debug=False,
                   num_devices=NCORES, dynamic_dma_scratch_size=65536)
    xT2 = nc.dram_tensor("xT2", [NTAB, ROWW], mybir.dt.bfloat16,
                         kind="ExternalInput")
    idx_d = nc.dram_tensor("idxall",
                           [P, 8 * (ngroups + len(sched["calls"]))],
                           mybir.dt.int16, kind="ExternalInput")
    valsw_d = nc.dram_tensor("valsw", [P, ngroups], mybir.dt.bfloat16,
                             kind="ExternalInput")
    biasp_d = nc.dram_tensor("biasp", [P, NBLK], mybir.dt.float32,
                             kind="ExternalInput")
    out_d = nc.dram_tensor("out", [NBLK, P, BATCH], mybir.dt.float32,
                           kind="ExternalOutput")

    with tile.TileContext(nc) as tc:
        with (
            tc.tile_pool(name="const", bufs=1) as const_pool,
            tc.tile_pool(name="idx", bufs=3) as idx_pool,
            tc.tile_pool(name="gat", bufs=3) as gat_pool,
            tc.tile_pool(name="scaled", bufs=2) as sc_pool,
            tc.tile_pool(name="psum", bufs=8, space="PSUM") as psum_pool,
            tc.tile_pool(name="outp", bufs=4) as out_pool,
        ):
            ident = const_pool.tile([P, P], mybir.dt.bfloat16)
            make_identity(nc, ident[:])
            valsw_t = const_pool.tile([P, ngroups], mybir.dt.bfloat16)
            nc.sync.dma_start(valsw_t[:], valsw_d[:])
            biasp_t = const_pool.tile([P, NBLK], mybir.dt.float32)
            nc.sync.dma_start(biasp_t[:], biasp_d[:])

            ps = {}
            for ci, (start, n, w) in enumerate(sched["calls"]):
                ioff = 8 * (start + ci)
                it = idx_pool.tile([P, 8 * (n + 1)], mybir.dt.int16)
                nc.sync.dma_start(it[:], idx_d[:, ioff:ioff + 8 * (n + 1)])
                gt = gat_pool.tile([P, (n + 1) * ROWW], mybir.dt.bfloat16)
                nc.gpsimd.dma_gather(
                    out_ap=gt[:].rearrange("p (c f) -> p c f", f=ROWW),
                    in_ap=xT2[bases[w]:, :],
                    idxs_ap=it[:],
                    num_idxs=(n + 1) * P,
                    num_idxs_reg=(n + 1) * P,
                    elem_size=ROWW,
                    single_packet=False,
                )
                st = sc_pool.tile([P, n * BATCH], mybir.dt.bfloat16)
                nc.vector.tensor_tensor(
                    out=st[:].rearrange("p (c f) -> p c f", f=BATCH),
                    in0=gt[:].rearrange(
                        "p (c f) -> p c f", f=ROWW)[:, :n, :BATCH],
                    in1=valsw_t[:, start:start + n, None].to_broadcast(
                        [P, n, BATCH]),
                    op=mybir.AluOpType.mult,
                )
                for gl in range(n):
                    g = start + gl
                    b = int(blk_of_g[g])
                    if g == first_of[b]:
                        ps[b] = psum_pool.tile([P, BATCH], mybir.dt.float32,
                                               space="PSUM", name="ps")
                    nc.tensor.matmul(
                        ps[b][:], ident[:],
                        st[:, gl * BATCH:(gl + 1) * BATCH],
                        start=(g == first_of[b]), stop=(g == last_of[b]))
                    if g == last_of[b]:
                        ot = out_pool.tile([P, BATCH], mybir.dt.float32)
                        nc.vector.tensor_scalar(
                            out=ot[:], in0=ps.pop(b)[:],
                            scalar1=biasp_t[:, b:b + 1],
                            scalar2=None, op0=mybir.AluOpType.add)
                        nc.sync.dma_start(out_d[b], ot[:])
    nc.compile()
    _BUILD_CACHE[key] = nc
    return nc


def _inmaps(x, bias, sched, per_core, order_dst):
    import ml_dtypes
    bf16 = ml_dtypes.bfloat16

    xT = np.asarray(x).astype(np.float32).T          # [NUM_SRC, BATCH]
    table = np.zeros((sched["NTAB"], ROWW), dtype=bf16)
    table[sched["rowof"], :BATCH] = xT.astype(bf16)

    bias = np.asarray(bias).astype(np.float32)
    biaspad = np.zeros(DTOT, dtype=np.float32)
    biaspad[:NUM_DST] = bias[order_dst]
    bias_blocks = biaspad.reshape(NBLK, NCORES, P)   # [bl, k, p]
    in_maps = []
    for k in range(NCORES):
        idxall, valsw = per_core[k]
        bp = np.ascontiguousarray(bias_blocks[:, k, :].T)   # [P, NBLK]
        in_maps.append({"xT2": table, "idxall": idxall, "valsw": valsw,
                        "biasp": bp})
    return in_maps


def kernel(x, vals, bias, row, col):
    from concourse.bass_utils import run_bass_kernel_spmd

    sched, per_core, order_dst = _preprocess(row, col, vals)
    nc = _build(sched)
    in_maps = _inmaps(x, bias, sched, per_core, order_dst)

    res = run_bass_kernel_spmd(nc, in_maps, list(range(NCORES)))

    out = np.empty((NUM_DST, BATCH), dtype=np.float32)
    for k in range(NCORES):
        rows = res.results[k]["out"].reshape(NBLK, P, BATCH)
        rk = (np.arange(NBLK)[:, None] * NCORES + k) * P + np.arange(P)[None, :]
        valid = rk < NUM_DST
        out[order_dst[rk[valid]]] = rows[valid]
    return np.ascontiguousarray(out.T)



# revision 8
# speedup vs baseline: 2.8065x; 2.8065x over previous
"""COO SpMM (GNN message passing) on 8 Trainium2 NeuronCores.

out[b, d] = sum_e vals[e] * x[b, col[e]]  for row[e]==d,  + bias[d]

Strategy (dst-sharded, batched dma_gather, bf16):
  - Sources are split into two windows (A/B) so edge indices fit the
    int16 index range of dma_gather (signed offsets from a per-call
    in_ap base give a 64K-row reach per window; rows at idx==-1 are
    reserved zero rows).
  - Dsts are ranked by (degree desc, A-count desc) so the 128-dst
    blocks have near-uniform per-window degree profiles; global block
    g -> core g%8 keeps the shared SPMD schedule's padding small.
  - Per local block bl: L_A[bl] columns for window-A edges and L_B[bl]
    for window-B (max over the 8 cores).  Columns are laid out in
    superblocks of SB blocks: [A-cols of SB blocks][B-cols of ...] so
    each window run is one dma_gather call (~100 columns = ~13K rows
    gathered per call; the ~1us SWDGE fixed cost is amortized 100x).
  - Gathered rows are 128 bf16 (= 256B, dma_gather's minimum; cols
    64..127 are zero).  One broadcast tensor_tensor per call scales
    all columns by their edge weights (bf16, 2x DVE).  Per column a
    bf16 identity matmul accumulates the scaled [128, 64] slice into
    the block's fp32 PSUM accumulator; block done -> bias add -> DMA.
  - Host: un-permute ranks, transpose back to [64, 100000] f32.
"""
import sys
import numpy as np

sys.path.insert(0, "/opt/trn_rl_repo")

NUM_SRC = 100000
NUM_DST = 100000
NNZ = 3200000
BATCH = 64
NCORES = 8
P = 128
ROWW = 128                       # bf16 elements per table row (256B)
NBLK = 98                        # local blocks per core
NBLK_G = NBLK * NCORES           # 784 global blocks
DTOT = NBLK_G * P                # 100352 ranked dst slots
SB = 6                           # blocks per superblock
MAXC = 96                        # max columns per dma_gather call
HALF = 32768


def _preprocess(row, col, vals):
    import ml_dtypes
    bf16 = ml_dtypes.bfloat16

    row = np.asarray(row).astype(np.int64)
    col = np.asarray(col).astype(np.int64)
    vals = np.asarray(vals).astype(np.float32)

    # --- source -> window (A=0 / B=1): greedy per-dst balance -------------
    deg0 = np.bincount(row, minlength=NUM_DST)
    o = np.argsort(col, kind="stable")
    dsts_by_src = row[o]
    sdeg = np.bincount(col, minlength=NUM_SRC)
    indptr = np.concatenate([[0], np.cumsum(sdeg)])
    order_src = np.argsort(-sdeg, kind="stable")
    ip0, ip1 = indptr[:-1], indptr[1:]
    D = np.zeros(NUM_DST, np.int32)
    wsrc = np.zeros(NUM_SRC, np.int8)
    for s in order_src:
        ds = dsts_by_src[ip0[s]:ip1[s]]
        if D[ds].sum() >= 0:
            wsrc[s] = 1
            D[ds] -= 1
        else:
            D[ds] += 1
    sgn = {0: 1, 1: -1}
    for _ in range(2):
        for s in order_src:
            ds = dsts_by_src[ip0[s]:ip1[s]]
            c = sgn[wsrc[s]]
            w_new = 1 if (D[ds].sum() - c * len(ds)) >= 0 else 0
            if w_new != wsrc[s]:
                D[ds] += sgn[w_new] - c
                wsrc[s] = w_new
    a_src = np.where(wsrc == 0)[0]
    b_src = np.where(wsrc == 1)[0]
    nA_src, nB_src = len(a_src), len(b_src)
    # window A rows [0, A_end) skipping reserved row HALF-1
    a_rows = np.arange(nA_src, dtype=np.int64)
    if nA_src > HALF - 1:
        a_rows[a_rows >= HALF - 1] += 1
    A_end = int(a_rows[-1]) + 1 if nA_src else 0
    base_A = HALF
    base_B = A_end + HALF
    b_rows = A_end + np.arange(nB_src, dtype=np.int64)
    if nB_src > HALF - 1:
        b_rows[b_rows >= base_B - 1] += 1
    B_end = int(b_rows[-1]) + 1 if nB_src else A_end
    NTAB = max(B_end, base_B + 1)
    rowof = np.empty(NUM_SRC, np.int64)
    rowof[a_src] = a_rows
    rowof[b_src] = b_rows

    # --- dst ranking ------------------------------------------------------
    deg = np.bincount(row, minlength=NUM_DST)
    w_e = wsrc[col].astype(np.int64)
    nAd = np.bincount(row[w_e == 0], minlength=NUM_DST)
    order_dst = np.lexsort((-nAd, -deg))          # rank -> dst id
    rank_of = np.empty(NUM_DST, np.int64)
    rank_of[order_dst] = np.arange(NUM_DST)

    # --- per-block column counts (shared schedule) ------------------------
    nApad = np.zeros(DTOT, np.int64)
    nApad[:NUM_DST] = nAd[order_dst]
    nBpad = np.zeros(DTOT, np.int64)
    nBpad[:NUM_DST] = (deg - nAd)[order_dst]
    L_A = nApad.reshape(NBLK_G, P).max(1).reshape(NBLK, NCORES).max(1)
    L_B = nBpad.reshape(NBLK_G, P).max(1).reshape(NBLK, NCORES).max(1)
    L_A[(L_A + L_B) == 0] = 1

    # --- column layout: per superblock [A-cols][B-cols] -------------------
    colbase_A = np.zeros(NBLK, np.int64)
    colbase_B = np.zeros(NBLK, np.int64)
    calls = []                                    # (start, ncols, window)

    def _add_calls(s, n, w):
        while n > 0:
            c = min(n, MAXC)
            calls.append((s, c, w))
            s += c
            n -= c

    cptr = 0
    for sb0 in range(0, NBLK, SB):
        bls = range(sb0, min(sb0 + SB, NBLK))
        sA = cptr
        for bl in bls:
            colbase_A[bl] = cptr
            cptr += int(L_A[bl])
        _add_calls(sA, cptr - sA, 0)
        sB = cptr
        for bl in bls:
            colbase_B[bl] = cptr
            cptr += int(L_B[bl])
        _add_calls(sB, cptr - sB, 1)
    ngroups = cptr

    blk_of_g = np.zeros(ngroups, np.int64)
    for bl in range(NBLK):
        blk_of_g[colbase_A[bl]:colbase_A[bl] + L_A[bl]] = bl
        blk_of_g[colbase_B[bl]:colbase_B[bl] + L_B[bl]] = bl
    first_of = np.where(L_A > 0, colbase_A, colbase_B)
    last_of = np.where(L_B > 0, colbase_B + L_B - 1, colbase_A + L_A - 1)

    # --- per-edge schedule ------------------------------------------------
    r = rank_of[row]
    key = r * 2 + w_e
    order = np.argsort(key, kind="stable")
    k_s = key[order]
    col_s = col[order]
    vals_s = vals[order]
    counts = np.bincount(k_s, minlength=DTOT * 2)
    starts = np.concatenate([[0], np.cumsum(counts)[:-1]])
    j_s = np.arange(len(k_s)) - starts[k_s]
    r_s = k_s // 2
    w_s = k_s % 2
    gb_s = r_s // P
    slot_s = r_s % P
    core_s = gb_s % NCORES
    bl_s = gb_s // NCORES
    g_s = np.where(w_s == 0, colbase_A[bl_s], colbase_B[bl_s]) + j_s
    idx_s = rowof[col_s] - np.where(w_s == 0, base_A, base_B)
    assert idx_s.min() >= -HALF and idx_s.max() < HALF and not (idx_s == -1).any()

    per_core = []
    for k in range(NCORES):
        m = core_s == k
        idxmat = np.zeros((P, ngroups), dtype=np.int16)
        valsw = np.zeros((P, ngroups), dtype=bf16)
        idxmat[slot_s[m], g_s[m]] = idx_s[m].astype(np.int16)
        valsw[slot_s[m], g_s[m]] = vals_s[m].astype(bf16)
        # wrap idx into the 16-partition dma_gather layout, per call;
        # append one idx=0 pad column per call — the ucode silently drops a
        # TRAILING run of negative indices, so the list must end >= 0.
        parts = []
        for (start, n, w) in calls:
            sub = np.concatenate(
                [idxmat[:, start:start + n], np.zeros((P, 1), np.int16)], axis=1)
            wrapped = sub.reshape(8, 16, n + 1).transpose(1, 2, 0).reshape(
                16, 8 * (n + 1))
            parts.append(np.tile(wrapped, (8, 1)))
        idxall = np.concatenate(parts, axis=1)
        per_core.append((idxall, valsw))

    sched = {
        "calls": tuple(calls),
        "ngroups": ngroups,
        "NTAB": NTAB,
        "base_A": base_A,
        "base_B": base_B,
        "blk_of_g": blk_of_g,
        "first_of": first_of,
        "last_of": last_of,
        "rowof": rowof,
    }
    return sched, per_core, order_dst


_BUILD_CACHE = {}


def _build(sched, reps=1):
    key = (sched["ngroups"], sched["NTAB"], sched["calls"], reps)
    if key in _BUILD_CACHE:
        return _BUILD_CACHE[key]
    import concourse.bacc as bacc
    import concourse.mybir as mybir
    import concourse.tile as tile
    from concourse.masks import make_identity

    ngroups = sched["ngroups"]
    NTAB = sched["NTAB"]
    bases = (sched["base_A"], sched["base_B"])
    blk_of_g = sched["blk_of_g"]
    first_of = sched["first_of"]
    last_of = sched["last_of"]

    nc = bacc.Bacc("TRN2", target_bir_lowering=False, debug=False,
                   num_devices=NCORES, dynamic_dma_scratch_size=65536,
                   num_swdge_queues=4)
    xT2 = nc.dram_tensor("xT2", [NTAB, ROWW], mybir.dt.bfloat16,
                         kind="ExternalInput")
    idx_d = nc.dram_tensor("idxall",
                           [P, 8 * (ngroups + len(sched["calls"]))],
                           mybir.dt.int16, kind="ExternalInput")
    valsw_d = nc.dram_tensor("valsw", [P, ngroups], mybir.dt.bfloat16,
                             kind="ExternalInput")
    biasp_d = nc.dram_tensor("biasp", [P, NBLK], mybir.dt.float32,
                             kind="ExternalInput")
    out_d = nc.dram_tensor("out", [NBLK, P, BATCH], mybir.dt.float32,
                           kind="ExternalOutput")

    with tile.TileContext(nc) as tc:
        with (
            tc.tile_pool(name="const", bufs=1) as const_pool,
            tc.tile_pool(name="idx", bufs=4) as idx_pool,
            tc.tile_pool(name="gat", bufs=4) as gat_pool,
            tc.tile_pool(name="scaled", bufs=2) as sc_pool,
            tc.tile_pool(name="psum", bufs=8, space="PSUM") as psum_pool,
            tc.tile_pool(name="outp", bufs=4) as out_pool,
        ):
            ident = const_pool.tile([P, P], mybir.dt.bfloat16)
            make_identity(nc, ident[:])
            valsw_t = const_pool.tile([P, ngroups], mybir.dt.bfloat16)
            nc.sync.dma_start(valsw_t[:], valsw_d[:])
            biasp_t = const_pool.tile([P, NBLK], mybir.dt.float32)
            nc.sync.dma_start(biasp_t[:], biasp_d[:])

            for _rep in range(reps):
                _emit_schedule(nc, sched, tc, ident, valsw_t, biasp_t,
                               idx_pool, gat_pool, sc_pool, psum_pool,
                               out_pool, xT2, idx_d, valsw_d, biasp_d, out_d)
    nc.compile()
    _BUILD_CACHE[key] = nc
    return nc


def _emit_schedule(nc, sched, tc, ident, valsw_t, biasp_t, idx_pool,
                   gat_pool, sc_pool, psum_pool, out_pool, xT2, idx_d,
                   valsw_d, biasp_d, out_d):
    import concourse.mybir as mybir

    bases = (sched["base_A"], sched["base_B"])
    blk_of_g = sched["blk_of_g"]
    first_of = sched["first_of"]
    last_of = sched["last_of"]
    if True:
        if True:
            ps = {}
            for ci, (start, n, w) in enumerate(sched["calls"]):
                ioff = 8 * (start + ci)
                it = idx_pool.tile([P, 8 * (n + 1)], mybir.dt.int16)
                nc.sync.dma_start(it[:], idx_d[:, ioff:ioff + 8 * (n + 1)])
                gt = gat_pool.tile([P, (n + 1) * ROWW], mybir.dt.bfloat16)
                nc.gpsimd.dma_gather(
                    out_ap=gt[:].rearrange("p (c f) -> p c f", f=ROWW),
                    in_ap=xT2[bases[w]:, :],
                    idxs_ap=it[:],
                    num_idxs=(n + 1) * P,
                    num_idxs_reg=(n + 1) * P,
                    elem_size=ROWW,
                    single_packet=False,
                    queue_num=ci % 4,
                )
                st = sc_pool.tile([P, n * BATCH], mybir.dt.bfloat16)
                nc.vector.tensor_tensor(
                    out=st[:].rearrange("p (c f) -> p c f", f=BATCH),
                    in0=gt[:].rearrange(
                        "p (c f) -> p c f", f=ROWW)[:, :n, :BATCH],
                    in1=valsw_t[:, start:start + n, None].to_broadcast(
                        [P, n, BATCH]),
                    op=mybir.AluOpType.mult,
                )
                for gl in range(n):
                    g = start + gl
                    b = int(blk_of_g[g])
                    if g == first_of[b]:
                        ps[b] = psum_pool.tile([P, BATCH], mybir.dt.float32,
                                               space="PSUM", name="ps")
                    nc.tensor.matmul(
                        ps[b][:], ident[:],
                        st[:, gl * BATCH:(gl + 1) * BATCH],
                        start=(g == first_of[b]), stop=(g == last_of[b]))
                    if g == last_of[b]:
                        ot = out_pool.tile([P, BATCH], mybir.dt.float32)
                        nc.vector.tensor_scalar(
                            out=ot[:], in0=ps.pop(b)[:],
                            scalar1=biasp_t[:, b:b + 1],
                            scalar2=None, op0=mybir.AluOpType.add)
                        nc.sync.dma_start(out_d[b], ot[:])


def _inmaps(x, bias, sched, per_core, order_dst):
    import ml_dtypes
    bf16 = ml_dtypes.bfloat16

    xT = np.asarray(x).astype(np.float32).T          # [NUM_SRC, BATCH]
    table = np.zeros((sched["NTAB"], ROWW), dtype=bf16)
    table[sched["rowof"], :BATCH] = xT.astype(bf16)

    bias = np.asarray(bias).astype(np.float32)
    biaspad = np.zeros(DTOT, dtype=np.float32)
    biaspad[:NUM_DST] = bias[order_dst]
    bias_blocks = biaspad.reshape(NBLK, NCORES, P)   # [bl, k, p]
    in_maps = []
    for k in range(NCORES):
        idxall, valsw = per_core[k]
        bp = np.ascontiguousarray(bias_blocks[:, k, :].T)   # [P, NBLK]
        in_maps.append({"xT2": table, "idxall": idxall, "valsw": valsw,
                        "biasp": bp})
    return in_maps


def kernel(x, vals, bias, row, col):
    from concourse.bass_utils import run_bass_kernel_spmd

    sched, per_core, order_dst = _preprocess(row, col, vals)
    nc = _build(sched)
    in_maps = _inmaps(x, bias, sched, per_core, order_dst)

    res = run_bass_kernel_spmd(nc, in_maps, list(range(NCORES)))

    out = np.empty((NUM_DST, BATCH), dtype=np.float32)
    for k in range(NCORES):
        rows = res.results[k]["out"].reshape(NBLK, P, BATCH)
        rk = (np.arange(NBLK)[:, None] * NCORES + k) * P + np.arange(P)[None, :]
        valid = rk < NUM_DST
        out[order_dst[rk[valid]]] = rows[valid]
    return np.ascontiguousarray(out.T)



# revision 9
# speedup vs baseline: 5.8313x; 2.0778x over previous
"""COO SpMM (GNN message passing) on 8 Trainium2 NeuronCores.

out[b, d] = sum_e vals[e] * x[b, col[e]]  for row[e]==d,  + bias[d]

Strategy (dst-sharded, batched dma_gather, bf16):
  - Sources are split into two windows (A/B) so edge indices fit the
    int16 index range of dma_gather (signed offsets from a per-call
    in_ap base give a 64K-row reach per window; rows at idx==-1 are
    reserved zero rows).
  - Dsts are ranked by (degree desc, A-count desc) so the 128-dst
    blocks have near-uniform per-window degree profiles; global block
    g -> core g%8 keeps the shared SPMD schedule's padding small.
  - Per local block bl: L_A[bl] columns for window-A edges and L_B[bl]
    for window-B (max over the 8 cores).  Columns are laid out in
    superblocks of SB blocks: [A-cols of SB blocks][B-cols of ...] so
    each window run is one dma_gather call (~100 columns = ~13K rows
    gathered per call; the ~1us SWDGE fixed cost is amortized 100x).
  - Gathered rows are 128 bf16 (= 256B, dma_gather's minimum; cols
    64..127 are zero).  One broadcast tensor_tensor per call scales
    all columns by their edge weights (bf16, 2x DVE).  Per column a
    bf16 identity matmul accumulates the scaled [128, 64] slice into
    the block's fp32 PSUM accumulator; block done -> bias add -> DMA.
  - Host: un-permute ranks, transpose back to [64, 100000] f32.
"""
import sys
import numpy as np

sys.path.insert(0, "/opt/trn_rl_repo")

NUM_SRC = 100000
NUM_DST = 100000
NNZ = 3200000
BATCH = 64
NCORES = 8
P = 128
ROWW = 128                       # bf16 elements per table row (256B)
NBLK = 98                        # local blocks per core
NBLK_G = NBLK * NCORES           # 784 global blocks
DTOT = NBLK_G * P                # 100352 ranked dst slots
SB = 6                           # blocks per superblock
MAXC = 64                        # max columns per dma_gather call
HALF = 32768


def _preprocess(row, col, vals):
    import ml_dtypes
    bf16 = ml_dtypes.bfloat16

    row = np.asarray(row).astype(np.int64)
    col = np.asarray(col).astype(np.int64)
    vals = np.asarray(vals).astype(np.float32)

    # --- source -> window (A=0 / B=1): greedy per-dst balance -------------
    deg0 = np.bincount(row, minlength=NUM_DST)
    o = np.argsort(col, kind="stable")
    dsts_by_src = row[o]
    sdeg = np.bincount(col, minlength=NUM_SRC)
    indptr = np.concatenate([[0], np.cumsum(sdeg)])
    order_src = np.argsort(-sdeg, kind="stable")
    ip0, ip1 = indptr[:-1], indptr[1:]
    D = np.zeros(NUM_DST, np.int32)
    wsrc = np.zeros(NUM_SRC, np.int8)
    for s in order_src:
        ds = dsts_by_src[ip0[s]:ip1[s]]
        if D[ds].sum() >= 0:
            wsrc[s] = 1
            D[ds] -= 1
        else:
            D[ds] += 1
    sgn = {0: 1, 1: -1}
    for _ in range(2):
        for s in order_src:
            ds = dsts_by_src[ip0[s]:ip1[s]]
            c = sgn[wsrc[s]]
            w_new = 1 if (D[ds].sum() - c * len(ds)) >= 0 else 0
            if w_new != wsrc[s]:
                D[ds] += sgn[w_new] - c
                wsrc[s] = w_new
    a_src = np.where(wsrc == 0)[0]
    b_src = np.where(wsrc == 1)[0]
    nA_src, nB_src = len(a_src), len(b_src)
    # window A rows [0, A_end) skipping reserved row HALF-1
    a_rows = np.arange(nA_src, dtype=np.int64)
    if nA_src > HALF - 1:
        a_rows[a_rows >= HALF - 1] += 1
    A_end = int(a_rows[-1]) + 1 if nA_src else 0
    base_A = HALF
    base_B = A_end + HALF
    b_rows = A_end + np.arange(nB_src, dtype=np.int64)
    if nB_src > HALF - 1:
        b_rows[b_rows >= base_B - 1] += 1
    B_end = int(b_rows[-1]) + 1 if nB_src else A_end
    NTAB = max(B_end, base_B + 1)
    rowof = np.empty(NUM_SRC, np.int64)
    rowof[a_src] = a_rows
    rowof[b_src] = b_rows

    # --- dst ranking ------------------------------------------------------
    deg = np.bincount(row, minlength=NUM_DST)
    w_e = wsrc[col].astype(np.int64)
    nAd = np.bincount(row[w_e == 0], minlength=NUM_DST)
    order_dst = np.lexsort((-deg, -nAd))          # rank -> dst id (nA primary packs blocks tighter)
    rank_of = np.empty(NUM_DST, np.int64)
    rank_of[order_dst] = np.arange(NUM_DST)

    # --- per-block column counts (shared schedule) ------------------------
    nApad = np.zeros(DTOT, np.int64)
    nApad[:NUM_DST] = nAd[order_dst]
    nBpad = np.zeros(DTOT, np.int64)
    nBpad[:NUM_DST] = (deg - nAd)[order_dst]
    L_A = nApad.reshape(NBLK_G, P).max(1).reshape(NBLK, NCORES).max(1)
    L_B = nBpad.reshape(NBLK_G, P).max(1).reshape(NBLK, NCORES).max(1)
    L_A[(L_A + L_B) == 0] = 1

    # --- column layout: per superblock [A-cols][B-cols] -------------------
    colbase_A = np.zeros(NBLK, np.int64)
    colbase_B = np.zeros(NBLK, np.int64)
    calls = []                                    # (start, ncols, window)

    def _add_calls(s, n, w):
        while n > 0:
            c = min(n, MAXC)
            calls.append((s, c, w))
            s += c
            n -= c

    cptr = 0
    for sb0 in range(0, NBLK, SB):
        bls = range(sb0, min(sb0 + SB, NBLK))
        sA = cptr
        for bl in bls:
            colbase_A[bl] = cptr
            cptr += int(L_A[bl])
        _add_calls(sA, cptr - sA, 0)
        sB = cptr
        for bl in bls:
            colbase_B[bl] = cptr
            cptr += int(L_B[bl])
        _add_calls(sB, cptr - sB, 1)
    ngroups = cptr

    blk_of_g = np.zeros(ngroups, np.int64)
    for bl in range(NBLK):
        blk_of_g[colbase_A[bl]:colbase_A[bl] + L_A[bl]] = bl
        blk_of_g[colbase_B[bl]:colbase_B[bl] + L_B[bl]] = bl
    first_of = np.where(L_A > 0, colbase_A, colbase_B)
    last_of = np.where(L_B > 0, colbase_B + L_B - 1, colbase_A + L_A - 1)

    # --- per-edge schedule ------------------------------------------------
    r = rank_of[row]
    key = r * 2 + w_e
    order = np.argsort(key, kind="stable")
    k_s = key[order]
    col_s = col[order]
    vals_s = vals[order]
    counts = np.bincount(k_s, minlength=DTOT * 2)
    starts = np.concatenate([[0], np.cumsum(counts)[:-1]])
    j_s = np.arange(len(k_s)) - starts[k_s]
    r_s = k_s // 2
    w_s = k_s % 2
    gb_s = r_s // P
    slot_s = r_s % P
    core_s = gb_s % NCORES
    bl_s = gb_s // NCORES
    g_s = np.where(w_s == 0, colbase_A[bl_s], colbase_B[bl_s]) + j_s
    idx_s = rowof[col_s] - np.where(w_s == 0, base_A, base_B)
    assert idx_s.min() >= -HALF and idx_s.max() < HALF and not (idx_s == -1).any()

    per_core = []
    for k in range(NCORES):
        m = core_s == k
        idxmat = np.zeros((P, ngroups), dtype=np.int16)
        valsw = np.zeros((P, ngroups), dtype=bf16)
        idxmat[slot_s[m], g_s[m]] = idx_s[m].astype(np.int16)
        valsw[slot_s[m], g_s[m]] = vals_s[m].astype(bf16)
        # wrap idx into the 16-partition dma_gather layout, per call;
        # append one idx=0 pad column per call — the ucode silently drops a
        # TRAILING run of negative indices, so the list must end >= 0.
        parts = []
        for (start, n, w) in calls:
            sub = np.concatenate(
                [idxmat[:, start:start + n], np.zeros((P, 1), np.int16)], axis=1)
            wrapped = sub.reshape(8, 16, n + 1).transpose(1, 2, 0).reshape(
                16, 8 * (n + 1))
            parts.append(np.tile(wrapped, (8, 1)))
        idxall = np.concatenate(parts, axis=1)
        per_core.append((idxall, valsw))

    sched = {
        "calls": tuple(calls),
        "ngroups": ngroups,
        "NTAB": NTAB,
        "base_A": base_A,
        "base_B": base_B,
        "blk_of_g": blk_of_g,
        "first_of": first_of,
        "last_of": last_of,
        "rowof": rowof,
    }
    return sched, per_core, order_dst


_BUILD_CACHE = {}


def _build(sched, reps=1):
    key = (sched["ngroups"], sched["NTAB"], sched["calls"], reps)
    if key in _BUILD_CACHE:
        return _BUILD_CACHE[key]
    import concourse.bacc as bacc
    import concourse.mybir as mybir
    import concourse.tile as tile
    from concourse.masks import make_identity

    ngroups = sched["ngroups"]
    NTAB = sched["NTAB"]
    bases = (sched["base_A"], sched["base_B"])
    blk_of_g = sched["blk_of_g"]
    first_of = sched["first_of"]
    last_of = sched["last_of"]

    nc = bacc.Bacc("TRN2", target_bir_lowering=False, debug=False,
                   num_devices=NCORES, dynamic_dma_scratch_size=65536,
                   num_swdge_queues=4)
    xT2 = nc.dram_tensor("xT2", [NTAB, ROWW], mybir.dt.bfloat16,
                         kind="ExternalInput")
    idx_d = nc.dram_tensor("idxall",
                           [P, 8 * (ngroups + len(sched["calls"]))],
                           mybir.dt.int16, kind="ExternalInput")
    valsw_d = nc.dram_tensor("valsw", [P, ngroups], mybir.dt.bfloat16,
                             kind="ExternalInput")
    biasp_d = nc.dram_tensor("biasp", [P, NBLK], mybir.dt.float32,
                             kind="ExternalInput")
    out_d = nc.dram_tensor("out", [NBLK, P, BATCH], mybir.dt.float32,
                           kind="ExternalOutput")

    with tile.TileContext(nc) as tc:
        with (
            tc.tile_pool(name="const", bufs=1) as const_pool,
            tc.tile_pool(name="idx", bufs=6) as idx_pool,
            tc.tile_pool(name="gat", bufs=6) as gat_pool,
            tc.tile_pool(name="scaled", bufs=2) as sc_pool,
            tc.tile_pool(name="psum", bufs=8, space="PSUM") as psum_pool,
            tc.tile_pool(name="outp", bufs=4) as out_pool,
        ):
            ident = const_pool.tile([P, P], mybir.dt.bfloat16)
            make_identity(nc, ident[:])
            valsw_t = const_pool.tile([P, ngroups], mybir.dt.bfloat16)
            nc.sync.dma_start(valsw_t[:], valsw_d[:])
            biasp_t = const_pool.tile([P, NBLK], mybir.dt.float32)
            nc.sync.dma_start(biasp_t[:], biasp_d[:])

            for _rep in range(reps):
                _emit_schedule(nc, sched, tc, ident, valsw_t, biasp_t,
                               idx_pool, gat_pool, sc_pool, psum_pool,
                               out_pool, xT2, idx_d, valsw_d, biasp_d, out_d)
    nc.compile()
    _BUILD_CACHE[key] = nc
    return nc


def _emit_schedule(nc, sched, tc, ident, valsw_t, biasp_t, idx_pool,
                   gat_pool, sc_pool, psum_pool, out_pool, xT2, idx_d,
                   valsw_d, biasp_d, out_d):
    import concourse.mybir as mybir

    bases = (sched["base_A"], sched["base_B"])
    blk_of_g = sched["blk_of_g"]
    first_of = sched["first_of"]
    last_of = sched["last_of"]
    if True:
        if True:
            ps = {}
            for ci, (start, n, w) in enumerate(sched["calls"]):
                ioff = 8 * (start + ci)
                it = idx_pool.tile([P, 8 * (n + 1)], mybir.dt.int16)
                nc.sync.dma_start(it[:], idx_d[:, ioff:ioff + 8 * (n + 1)])
                gt = gat_pool.tile([P, (n + 1) * ROWW], mybir.dt.bfloat16)
                nc.gpsimd.dma_gather(
                    out_ap=gt[:].rearrange("p (c f) -> p c f", f=ROWW),
                    in_ap=xT2[bases[w]:, :],
                    idxs_ap=it[:],
                    num_idxs=(n + 1) * P,
                    num_idxs_reg=(n + 1) * P,
                    elem_size=ROWW,
                    single_packet=False,
                    queue_num=ci % 4,
                )
                st = sc_pool.tile([P, n * BATCH], mybir.dt.bfloat16)
                nc.vector.tensor_tensor(
                    out=st[:].rearrange("p (c f) -> p c f", f=BATCH),
                    in0=gt[:].rearrange(
                        "p (c f) -> p c f", f=ROWW)[:, :n, :BATCH],
                    in1=valsw_t[:, start:start + n, None].to_broadcast(
                        [P, n, BATCH]),
                    op=mybir.AluOpType.mult,
                )
                for gl in range(n):
                    g = start + gl
                    b = int(blk_of_g[g])
                    if g == first_of[b]:
                        ps[b] = psum_pool.tile([P, BATCH], mybir.dt.float32,
                                               space="PSUM", name="ps")
                    nc.tensor.matmul(
                        ps[b][:], ident[:],
                        st[:, gl * BATCH:(gl + 1) * BATCH],
                        start=(g == first_of[b]), stop=(g == last_of[b]))
                    if g == last_of[b]:
                        ot = out_pool.tile([P, BATCH], mybir.dt.float32)
                        nc.vector.tensor_scalar(
                            out=ot[:], in0=ps.pop(b)[:],
                            scalar1=biasp_t[:, b:b + 1],
                            scalar2=None, op0=mybir.AluOpType.add)
                        nc.sync.dma_start(out_d[b], ot[:])


def _inmaps(x, bias, sched, per_core, order_dst):
    import ml_dtypes
    bf16 = ml_dtypes.bfloat16

    xT = np.asarray(x).astype(np.float32).T          # [NUM_SRC, BATCH]
    table = np.zeros((sched["NTAB"], ROWW), dtype=bf16)
    table[sched["rowof"], :BATCH] = xT.astype(bf16)

    bias = np.asarray(bias).astype(np.float32)
    biaspad = np.zeros(DTOT, dtype=np.float32)
    biaspad[:NUM_DST] = bias[order_dst]
    bias_blocks = biaspad.reshape(NBLK, NCORES, P)   # [bl, k, p]
    in_maps = []
    for k in range(NCORES):
        idxall, valsw = per_core[k]
        bp = np.ascontiguousarray(bias_blocks[:, k, :].T)   # [P, NBLK]
        in_maps.append({"xT2": table, "idxall": idxall, "valsw": valsw,
                        "biasp": bp})
    return in_maps


def kernel(x, vals, bias, row, col):
    from concourse.bass_utils import run_bass_kernel_spmd

    sched, per_core, order_dst = _preprocess(row, col, vals)
    nc = _build(sched)
    in_maps = _inmaps(x, bias, sched, per_core, order_dst)

    res = run_bass_kernel_spmd(nc, in_maps, list(range(NCORES)))

    out = np.empty((NUM_DST, BATCH), dtype=np.float32)
    for k in range(NCORES):
        rows = res.results[k]["out"].reshape(NBLK, P, BATCH)
        rk = (np.arange(NBLK)[:, None] * NCORES + k) * P + np.arange(P)[None, :]
        valid = rk < NUM_DST
        out[order_dst[rk[valid]]] = rows[valid]
    return np.ascontiguousarray(out.T)

